# revision 30
# baseline (speedup 1.0000x reference)
"""Trainium2 Bass kernel for nn_BasicLayer (sparse cluster attention, 2 layers).

v6 (final): slot-scheduled pipeline + compact block-diagonal softmax + bf16
residual stream.  854,989 ns (staged baseline) -> 750,282 ns measured.

Host side: scanline (boustrophedon) gather into cluster order, data-parallel
over 8 cores (8192 tokens each, 16 supertiles of 512), LN affine + all biases
folded into bf16 matmul weights shipped as two packed blobs (biases are zero
for this problem; asserted).  x/y travel as bf16 (halves HBM traffic; rel err
3.7e-3 vs the 2e-2 gate).

On-device vs the staged v2 baseline:
- Scores are computed block-diagonally: two matmuls per head (one per
  64-token cluster, output partition ranges 0:64/64:128, col-tiled), each
  head in its own PSUM bank slot so the four concurrent row-group matmuls
  never share a bank (sharing one hangs the device).  This makes every
  softmax element valid: exp runs as two full-width ACT calls into a compact
  E[128,2,6,64], one full-width DVE reduce per sub, bf16 reciprocal, and the
  P multiply splits DVE/GpSimd halves into persistent zero-initialized P
  buffers (off-diagonal stays zero) that DMA-transpose whole, so the O
  matmuls are unchanged.
- Residual stream x is bf16 (frees 4KB/partition SBUF, 4x DVE tier for the
  LN normalize); LN1 normalize runs on the Scalar engine (Identity with
  per-partition scale/bias - identity is resident in every ACT table set so
  it costs no table switches), LN2 normalize on DVE; the fast-inverse-sqrt
  chain's tensor_tensor steps run on GpSimd.
- Emission is slot-scheduled: super-units (layer, half-of-tiles) with phases
  LN1/QKV/CH/PROJ+LN2/MLP at offsets 5*layer + 4*half, so DVE-heavy phases
  (LN, softmax) share slots with PE-heavy phases (MLP, QKV) of other units,
  thunk lists round-robin-interleaved within each slot.  This keeps all
  engines co-active, batches exp vs gelu ACT-table usage (~10 table loads vs
  50 for a naive interleave), and keeps the PE dense enough to limit HAM
  half-clock throttling.
- oPp has its own PSUM pool: sharing the "med" PSUM tag between the O
  matmuls and qkv/proj/fc produced a cross-pool slot cycle (phase_a waits on
  a v_tm slot freed by attn_out which waits on a PSUM slot held by phase_a)
  that Tile's scheduler simulation flags as a deadlock.
"""

import os
import numpy as np
import ml_dtypes

# ---- problem constants (hardcoded per contract) ----
B, N, D = 4, 16384, 192
DP = 256
HEADS, DH, CLM = 6, 32, 64
GRID_W = 128
DEPTH = 2
NCORES = 8
T = (B * N) // NCORES                # 8192 tokens per core
SUB = 128
NSUB = 4
TILE = SUB * NSUB                    # 512-token supertile
NTILES = T // TILE                   # 16
GPB = 4                              # tiles per pipeline group
DFF = 768

# packed weight blob column layout (per layer)
W0_COLS = 384 + 192 + 192 + 768 + 1152   # 2688
W1_COLS = 384 + 192 + 192 + 768          # 1536

_COMPILED = {}


def _scanline_order(pos, w):
    ix = np.floor(pos[..., 0]).astype(np.int64)
    iy = np.floor(pos[..., 1]).astype(np.int64)
    key = iy * w + np.where(iy % 2 == 1, w - 1 - ix, ix)
    return np.argsort(key, axis=1, kind="stable")


def _fold_weights(inputs):
    """Fold LN affine + biases into matmul weights; pack into two blobs."""
    bf16 = ml_dtypes.bfloat16
    scale = DH ** -0.5
    wb0 = np.zeros((128, DEPTH * W0_COLS), np.float64)
    wb1 = np.zeros((64, DEPTH * W1_COLS), np.float64)
    bias_norm = 0.0
    for i in range(DEPTH):
        g1 = np.asarray(inputs["ln1_g"][i], np.float64)
        b1 = np.asarray(inputs["ln1_b"][i], np.float64)
        Wqkv = np.asarray(inputs["w_qkv"][i], np.float64)
        bqkv = np.asarray(inputs["b_qkv"][i], np.float64)
        w_eff = g1[:, None] * Wqkv
        b_eff = b1 @ Wqkv + bqkv
        wq = w_eff[:, 0:D] * scale
        wk = w_eff[:, D:2 * D]
        wv = w_eff[:, 2 * D:3 * D]
        wqk = np.concatenate(
            [wq[:, :128], wk[:, :128], wq[:, 128:], wk[:, 128:]], axis=1)
        wp = np.asarray(inputs["w_proj"][i], np.float64)
        bp = np.asarray(inputs["b_proj"][i], np.float64)
        g2 = np.asarray(inputs["ln2_g"][i], np.float64)
        b2 = np.asarray(inputs["ln2_b"][i], np.float64)
        W1 = np.asarray(inputs["w_fc1"][i], np.float64)
        w1_eff = g2[:, None] * W1
        b1_eff = b2 @ W1 + np.asarray(inputs["b_fc1"][i], np.float64)
        W2 = np.asarray(inputs["w_fc2"][i], np.float64)
        bfc2 = np.asarray(inputs["b_fc2"][i], np.float64)
        bias_norm += (np.abs(b_eff).sum() + np.abs(bp).sum()
                      + np.abs(b1_eff).sum() + np.abs(bfc2).sum())
        w2m = W2.reshape(6, 128, D).transpose(1, 0, 2).reshape(128, 6 * D)
        c0 = i * W0_COLS
        wb0[:, c0:c0 + 384] = wqk[0:128]
        wb0[:, c0 + 384:c0 + 576] = wv[0:128]
        wb0[:, c0 + 576:c0 + 768] = wp[0:128]
        wb0[:, c0 + 768:c0 + 1536] = w1_eff[0:128]
        wb0[:, c0 + 1536:c0 + 2688] = w2m
        c1 = i * W1_COLS
        wb1[:, c1:c1 + 384] = wqk[128:192]
        wb1[:, c1 + 384:c1 + 576] = wv[128:192]
        wb1[:, c1 + 576:c1 + 768] = wp[128:192]
        wb1[:, c1 + 768:c1 + 1536] = w1_eff[128:192]
    if bias_norm > 1e-12:
        raise NotImplementedError(
            "kernel v3 assumes all folded biases are zero "
            "(true for this problem's setup_inputs)")
    return {"wb0": wb0.astype(bf16), "wb1": wb1.astype(bf16)}


def _build_nc(ntiles=NTILES):
    key = ("nc", ntiles, os.environ.get("K_NO_GPS"), os.environ.get("K_RECIP_F32"), os.environ.get("K_SEQ"), os.environ.get("K_BASE_SOFT"))
    if key in _COMPILED:
        return _COMPILED[key]

    from contextlib import ExitStack
    import concourse.bass as bass
    import concourse.tile as tile
    from concourse import bacc, mybir
    from concourse.bass import ts, ds

    f32 = mybir.dt.float32
    bf16 = mybir.dt.bfloat16
    i32 = mybir.dt.int32
    AF = mybir.ActivationFunctionType
    OP = mybir.AluOpType
    AX = mybir.AxisListType

    nc = bacc.Bacc("TRN2", target_bir_lowering=False, debug=False,
                   enable_asserts=False, num_devices=NCORES)

    x_d = nc.dram_tensor("x", [ntiles, 128, NSUB * DP], bf16,
                         kind="ExternalInput").ap()
    y_d = nc.dram_tensor("y", [ntiles, 128, NSUB * D], bf16,
                         kind="ExternalOutput").ap()
    wb0_d = nc.dram_tensor("wb0", [128, DEPTH * W0_COLS], bf16,
                           kind="ExternalInput").ap()
    wb1_d = nc.dram_tensor("wb1", [64, DEPTH * W1_COLS], bf16,
                           kind="ExternalInput").ap()

    with tile.TileContext(nc) as tc, ExitStack() as ctx:
        consts = ctx.enter_context(tc.tile_pool(name="consts", bufs=1))
        xpool = ctx.enter_context(tc.tile_pool(name="xpool", bufs=16))
        lnpool = ctx.enter_context(tc.tile_pool(name="lnpool", bufs=6))
        fmpool = ctx.enter_context(tc.tile_pool(name="fmpool", bufs=12))
        qkpool = ctx.enter_context(tc.tile_pool(name="qkpool", bufs=9))
        epool = ctx.enter_context(tc.tile_pool(name="epool", bufs=6))
        ofpool = ctx.enter_context(tc.tile_pool(name="ofpool", bufs=10))
        hpool = ctx.enter_context(tc.tile_pool(name="hpool", bufs=2))
        stpool = ctx.enter_context(tc.tile_pool(name="stpool", bufs=10))
        ppsc = ctx.enter_context(tc.tile_pool(name="ppsc", bufs=1, space="PSUM"))
        ppm = ctx.enter_context(tc.tile_pool(name="ppm", bufs=3, space="PSUM"))
        ppo = ctx.enter_context(tc.tile_pool(name="ppo", bufs=1, space="PSUM"))

        # persistent zeroed P buffers: off-diagonal blocks stay 0 forever
        NPK = 4
        pk_bufs = []
        for pb_i in range(NPK):
            pb = consts.tile([128, 2, HEADS, 128], bf16, name=f"pkbuf{pb_i}")
            nc.vector.memset(pb, 0.0)
            pk_bufs.append(pb)
        pk_ctr = [0]
        pkpool = ctx.enter_context(tc.tile_pool(name="pkpool", bufs=3))

        # --- packed weights, two DMAs ---
        wb0_t = consts.tile([128, DEPTH * W0_COLS], bf16, name="wb0")
        wb1_t = consts.tile([64, DEPTH * W1_COLS], bf16, name="wb1")
        nc.scalar.dma_start(out=wb0_t, in_=wb0_d)
        nc.scalar.dma_start(out=wb1_t, in_=wb1_d)
        W = []
        for i in range(DEPTH):
            c0 = i * W0_COLS
            c1 = i * W1_COLS
            W.append({
                "wqk0": wb0_t[:, c0:c0 + 384],
                "wv0": wb0_t[:, c0 + 384:c0 + 576],
                "wp0": wb0_t[:, c0 + 576:c0 + 768],
                "w10": wb0_t[:, c0 + 768:c0 + 1536],
                "w2m": wb0_t[:, c0 + 1536:c0 + 2688].rearrange(
                    "p (m n) -> p m n", m=6),
                "wqk1": wb1_t[:, c1:c1 + 384],
                "wv1": wb1_t[:, c1 + 384:c1 + 576],
                "wp1": wb1_t[:, c1 + 576:c1 + 768],
                "w11": wb1_t[:, c1 + 768:c1 + 1536],
            })

        MAGIC = 0x5F3759DF
        # CoreSim lacks Gelu_apprx_tanh; substitute Tanh for sim-only runs.
        GELU_FUNC = (AF.Tanh if os.environ.get("K_SIM_GELU_TANH") == "1"
                     else AF.Gelu_apprx_tanh)

        def load_x(it):
            x_t = xpool.tile([128, NSUB, DP], bf16, tag="x", name=f"x{it}")
            nc.sync.dma_start(
                out=x_t,
                in_=x_d[it].rearrange("p (s f) -> p s f", s=NSUB))
            return x_t

        def layernorm_fm(x_t, norm_on_act):
            """LN on token-major x_t -> feature-major bf16 via DMA transpose."""
            mv = stpool.tile([128, NSUB, 6], f32, tag="mv", name="mv")
            mv2 = stpool.tile([128, NSUB, 2], f32, tag="mv2", name="mv2")
            for s in range(NSUB):
                nc.vector.bn_stats(mv[:, s], x_t[:, s, 0:D])
                nc.vector.bn_aggr(mv2[:, s], mv[:, s])
            mean = mv2[:, :, 0]                      # [128, 4] stride 2
            var = mv2[:, :, 1]
            t_i = stpool.tile([128, NSUB], i32, tag="ti", name="t_i")
            y0 = stpool.tile([128, NSUB], f32, tag="y0", name="y0")
            zz = stpool.tile([128, NSUB], f32, tag="zz", name="zz")
            r4 = stpool.tile([128, NSUB], f32, tag="r4", name="r4")
            g = nc.vector if os.environ.get('K_NO_GPS') == '1' else nc.gpsimd
            nc.vector.tensor_scalar(
                out=t_i, in0=var.bitcast(i32), scalar1=1, scalar2=None,
                op0=OP.logical_shift_right)
            nc.vector.tensor_scalar(
                out=y0.bitcast(i32), in0=t_i, scalar1=MAGIC, scalar2=-1,
                op0=OP.subtract, op1=OP.mult)
            nc.vector.scalar_tensor_tensor(
                out=zz, in0=var, scalar=1e-5, in1=y0,
                op0=OP.add, op1=OP.mult)              # (var+eps)*y0
            g.tensor_tensor(out=zz, in0=zz, in1=y0, op=OP.mult)
            nc.vector.tensor_scalar(
                out=zz, in0=zz, scalar1=-0.5, scalar2=1.5,
                op0=OP.mult, op1=OP.add)              # 1.5 - 0.5 v y0^2
            g.tensor_tensor(out=r4, in0=zz, in1=y0, op=OP.mult)

            xn = lnpool.tile([128, 2, NSUB, 128], bf16, tag="xn", name="xn")
            if norm_on_act:
                mr = stpool.tile([128, NSUB], f32, tag="mr", name="mr")
                nmr = stpool.tile([128, NSUB], f32, tag="nmr", name="nmr")
                g.tensor_tensor(out=mr, in0=mean, in1=r4, op=OP.mult)
                nc.vector.tensor_scalar(
                    out=nmr, in0=mr, scalar1=-1.0, scalar2=None,
                    op0=OP.mult)                      # -mean*r
                for s in range(NSUB):
                    nc.scalar.activation(
                        xn[:, :, s],
                        x_t[:, s].rearrange("p (c f) -> p c f", c=2),
                        AF.Identity,
                        bias=nmr[:, s:s + 1], scale=r4[:, s:s + 1])
            else:
                for s in range(NSUB):
                    nc.vector.tensor_scalar(
                        out=xn[:, :, s],
                        in0=x_t[:, s].rearrange("p (c f) -> p c f", c=2),
                        scalar1=mean[:, s:s + 1], scalar2=r4[:, s:s + 1],
                        op0=OP.subtract, op1=OP.mult)
            fm2 = fmpool.tile([128, 2, NSUB, 128], bf16, tag="fm", name="fm")
            nc.sync.dma_start_transpose(out=fm2, in_=xn)
            return fm2[:, 0], fm2[:, 1]

        def phase_a(sb, fmA, fmA2):
            """qkv + v from feature-major LN output. Returns (qkA, qkB, v_tm)."""
            fmAf = fmA.rearrange("p a b -> p (a b)")
            fmA2f = fmA2.rearrange("p a b -> p (a b)")
            qkA = qkpool.tile([128, 2, TILE], bf16, tag="qkA", name="qkA")
            qkB = qkpool.tile([64, 2, TILE], bf16, tag="qkB", name="qkB")
            for m in range(2):
                ps = ppm.tile([128, TILE], f32, tag="med", name=f"psqA{m}")
                nc.tensor.matmul(ps, sb["wqk0"][:, ts(m, 128)], fmAf,
                                 start=True, stop=False)
                nc.tensor.matmul(ps, sb["wqk1"][:, ts(m, 128)],
                                 fmA2f[0:64], start=False, stop=True)
                nc.scalar.activation(qkA[:, m], ps, AF.Copy)
            for m in range(2):
                ps = ppm.tile([64, TILE], f32, tag="med", name=f"psqB{m}")
                nc.tensor.matmul(ps, sb["wqk0"][:, ds(256 + m * 64, 64)],
                                 fmAf, start=True, stop=False)
                nc.tensor.matmul(ps, sb["wqk1"][:, ds(256 + m * 64, 64)],
                                 fmA2f[0:64], start=False, stop=True)
                nc.scalar.activation(qkB[:, m], ps, AF.Copy)
            v_tm = qkpool.tile([128, NSUB, D], bf16, tag="vtm", name="v_tm")
            for sp in range(2):
                psv = ppm.tile([128, 2, 256], f32, tag="med", name="psv")
                for j in range(2):
                    s = sp * 2 + j
                    nc.tensor.matmul(psv[:, j, 0:D], fmA[:, s], sb["wv0"],
                                     start=True, stop=False)
                    nc.tensor.matmul(psv[:, j, 0:D], fmA2[0:64, s], sb["wv1"],
                                     start=False, stop=True)
                nc.scalar.activation(v_tm[:, ds(sp * 2, 2)],
                                     psv[:, :, 0:D], AF.Copy)
            return qkA, qkB, v_tm

        def attn_soft(qkA, qkB, sp):
            """Compact block-diagonal scores + softmax + P^T for one sub-pair.

            Scores are computed with two matmuls per head (one per 64-token
            cluster, output partition ranges 0:64 / 64:128) into a compact
            all-valid [128, h, 64] layout, so exp / reduce / recip run at
            full partition width with no wasted work.  Head h lives at
            (m=h%4, 64-col slot h//4) so the four concurrently-running
            row-group matmuls land in different PSUM banks.
            """
            E2 = epool.tile([128, 2, HEADS, 64], bf16, tag="E2", name="E2")
            P2 = pk_bufs[pk_ctr[0] % NPK]
            pk_ctr[0] += 1
            for j in range(2):
                s = sp * 2 + j
                sc = ppsc.tile([128, 4, 512], f32, tag="sc", name="sc")
                scv = sc.rearrange("p m (g f) -> p g m f", g=8)
                for h in range(HEADS):
                    if h < 4:
                        qrow, kq = ts(h, 32), qkA
                        mslot, gslot = h, 0
                    else:
                        qrow, kq = ts(h - 4, 32), qkB
                        mslot, gslot = h - 4, 1
                    for c in range(2):
                        ccols = ds(s * 128 + c * 64, 64)
                        nc.tensor.matmul(
                            sc[ds(c * 64, 64), mslot, ds(gslot * 64, 64)],
                            kq[qrow, 0, ccols], kq[qrow, 1, ccols],
                            start=True, stop=True,
                            tile_position=(32 * mslot, c * 64))
                nc.scalar.activation(E2[:, j, 0:4], scv[:, 0], AF.Exp)
                nc.scalar.activation(E2[:, j, 4:6], scv[:, 1, 0:2], AF.Exp)
                sums = stpool.tile([128, HEADS], f32, tag="sm", name="sums")
                rsum = stpool.tile([128, HEADS], bf16, tag="rs", name="rsum")
                nc.vector.reduce_sum(sums, E2[:, j], axis=AX.X)
                with nc.allow_low_precision(
                        reason="bf16 softmax reciprocal is well within "
                               "the 2e-2 relative error budget"):
                    nc.vector.reciprocal(rsum, sums)
                for half in range(2):
                    hs = ds(half * 64, 64)
                    rs_half = rsum[hs]
                    rsum_b = bass.AP(tensor=rs_half.tensor,
                                     offset=rs_half.offset,
                                     ap=[*rs_half.ap, [0, 64]])
                    nc.gpsimd.tensor_tensor(
                        out=P2[hs, j, :, ds(half * 64, 64)],
                        in0=E2[hs, j], in1=rsum_b, op=OP.mult)
            pkm2 = pkpool.tile([128, 2, HEADS, 128], bf16, tag="pkm",
                               name="pkm")
            nc.sync.dma_start_transpose(out=pkm2, in_=P2)
            return pkm2

        def attn_out(v_tm, pk2, ofmA, ofmB, sp):
            """attention O matmuls + feature-major output copies."""
            oPp = ppo.tile([128, 2, 256], f32, tag="oPp", name="oPp")
            for j in range(2):
                s = sp * 2 + j
                for h in range(HEADS):
                    if h < 4:
                        out = oPp[ts(h, 32), j, 0:128]
                        colpos = h * 32
                    else:
                        out = oPp[ts(h - 4, 32), j, 128:256]
                        colpos = (h - 4) * 32
                    nc.tensor.matmul(out, v_tm[:, s, ts(h, 32)],
                                     pk2[:, j, h], start=True, stop=True,
                                     tile_position=(0, colpos))
            nc.vector.tensor_copy(
                ofmA.rearrange("p (a b) -> p a b", a=NSUB)[:, ds(sp * 2, 2)],
                oPp[:, :, 0:128])
            nc.vector.tensor_copy(
                ofmB.rearrange("p (a b) -> p a b", a=NSUB)[:, ds(sp * 2, 2)],
                oPp[0:64, :, 128:256])

        def phase_c_proj(sb, x_t, ofmA, ofmB):
            for sp in range(2):
                psp = ppm.tile([128, 2, 256], f32, tag="med", name="psp")
                for j in range(2):
                    s = sp * 2 + j
                    nc.tensor.matmul(psp[:, j, 0:D], ofmA[:, ts(s, 128)],
                                     sb["wp0"], start=True, stop=False)
                    nc.tensor.matmul(psp[:, j, 0:D], ofmB[:, ts(s, 128)],
                                     sb["wp1"], start=False, stop=True)
                nc.vector.tensor_add(x_t[:, ds(sp * 2, 2), 0:D],
                                     x_t[:, ds(sp * 2, 2), 0:D],
                                     psp[:, :, 0:D])

        def phase_c_mlp(sb, x_t, ynA, ynA2):
            ynAf = ynA.rearrange("p a b -> p (a b)")
            ynA2f = ynA2.rearrange("p a b -> p (a b)")
            hfm = hpool.tile([128, 6, TILE], bf16, tag="hfm", name="hfm")
            for m in range(6):
                psf = ppm.tile([128, TILE], f32, tag="med", name="psf1")
                nc.tensor.matmul(psf, sb["w10"][:, ts(m, 128)],
                                 ynAf, start=True, stop=False)
                nc.tensor.matmul(psf, sb["w11"][:, ts(m, 128)],
                                 ynA2f[0:64], start=False, stop=True)
                nc.scalar.activation(hfm[:, m], psf, GELU_FUNC)
            for sp in range(2):
                psf2 = ppm.tile([128, 2, 256], f32, tag="med", name="psf2")
                for j in range(2):
                    s = sp * 2 + j
                    for m in range(6):
                        nc.tensor.matmul(psf2[:, j, 0:D],
                                         hfm[:, m, ts(s, 128)],
                                         sb["w2m"][:, m],
                                         start=(m == 0), stop=(m == 5))
                nc.vector.tensor_add(x_t[:, ds(sp * 2, 2), 0:D],
                                     x_t[:, ds(sp * 2, 2), 0:D],
                                     psf2[:, :, 0:D])

        def store_y(it, x_t):
            nc.sync.dma_start(
                out=y_d[it].rearrange("p (s f) -> p s f", s=NSUB),
                in_=x_t[:, :, 0:D])

        # ---- slot-scheduled emission (v4 structure) ----
        # Five phases per super-unit (layer, half-of-tiles):
        # LN1 / QKV / CH / PROJ+LN2 / MLP, with unit offsets chosen so
        # DVE-heavy phases share slots with PE-heavy phases of other units
        # and exp/gelu stay batched per slot.
        GPB2 = min(8, ntiles)
        nstream = ntiles // GPB2
        US = []
        for li in range(DEPTH):
            for s in range(nstream):
                US.append((li, [s * GPB2 + t for t in range(GPB2)]))
        offs = []
        for ui, (li, _) in enumerate(US):
            s = ui % nstream if nstream > 1 else 0
            offs.append(5 * li + 4 * s)

        xts = {}
        fms = {}
        qks = {}
        ofs_ = {}
        yns = {}

        def ln1_thunks(u):
            li, tiles = u
            out = []
            for it in tiles:
                def th(li=li, it=it):
                    fms[(li, it)] = layernorm_fm(xts[it], norm_on_act=True)
                out.append(th)
            return out

        def qkv_thunks(u):
            li, tiles = u
            out = []
            for it in tiles:
                def th(li=li, it=it):
                    qks[(li, it)] = phase_a(W[li], *fms.pop((li, it)))
                out.append(th)
            return out

        def ch_thunks(u):
            li, tiles = u
            thunks = []
            for it in tiles:
                def alloc(li=li, it=it):
                    ofs_[(li, it)] = (
                        ofpool.tile([128, TILE], bf16, tag="ofA",
                                    name="ofmA"),
                        ofpool.tile([64, TILE], bf16, tag="ofB",
                                    name="ofmB"))
                thunks.append(alloc)
            chains = [(it, sp) for it in tiles for sp in range(2)]
            SKEW = 2
            pks = {}
            for ci in range(len(chains) + SKEW):
                if ci < len(chains):
                    def soft(ci=ci, li=li):
                        it, sp = chains[ci]
                        qkA, qkB, _ = qks[(li, it)]
                        pks[ci] = attn_soft(qkA, qkB, sp)
                    thunks.append(soft)
                if ci >= SKEW:
                    def outt(ci=ci, li=li):
                        it, sp = chains[ci - SKEW]
                        _, _, v_tm = qks[(li, it)]
                        attn_out(v_tm, pks.pop(ci - SKEW),
                                 ofs_[(li, it)][0], ofs_[(li, it)][1], sp)
                    thunks.append(outt)
            def drop(li=li, tiles=tiles):
                for it in tiles:
                    qks.pop((li, it))
            thunks.append(drop)
            return thunks

        def projln2_thunks(u):
            li, tiles = u
            n = len(tiles)
            thunks = []
            for i in range(n + 1):
                if i < n:
                    it = tiles[i]
                    def proj(li=li, it=it):
                        ofmA, ofmB = ofs_.pop((li, it))
                        phase_c_proj(W[li], xts[it], ofmA, ofmB)
                    thunks.append(proj)
                if 0 <= i - 1 < n:
                    it2 = tiles[i - 1]
                    def ln2(li=li, it=it2):
                        yns[(li, it)] = layernorm_fm(xts[it],
                                                     norm_on_act=False)
                    thunks.append(ln2)
            return thunks

        def mlp_thunks(u):
            li, tiles = u
            out = []
            for it in tiles:
                def th(li=li, it=it):
                    phase_c_mlp(W[li], xts[it], *yns.pop((li, it)))
                    if li == DEPTH - 1:
                        store_y(it, xts[it])
                out.append(th)
            return out

        PHASES = [ln1_thunks, qkv_thunks, ch_thunks, projln2_thunks,
                  mlp_thunks]

        from collections import defaultdict as _dd
        slots = _dd(list)
        for ui, u in enumerate(US):
            for p in range(5):
                slots[offs[ui] + p].append((p, u))

        for it in range(ntiles):
            xts[it] = load_x(it)

        def interleave(lists):
            idx = [0] * len(lists)
            total = sum(len(l) for l in lists)
            for _ in range(total):
                best, bf = None, 2.0
                for li_, l in enumerate(lists):
                    if idx[li_] < len(l):
                        f = idx[li_] / len(l)
                        if f < bf:
                            best, bf = li_, f
                lists[best][idx[best]]()
                idx[best] += 1

        for sl in sorted(slots):
            entries = slots[sl]
            entries.sort(key=lambda e: (e[0] != 2, e[0]))
            interleave([PHASES[p](u) for p, u in entries])

    nc.compile()
    _COMPILED[key] = nc
    return nc


def _ensure_ntff_hook():
    import sys, types
    if "antenv.axon_hooks" in sys.modules:
        return True
    try:
        mod = types.ModuleType("antenv.axon_hooks")
        state = {}
        mod.set_axon_ntff_profile_hook = lambda h: state.__setitem__("h", h)
        mod.get_axon_ntff_profile_hook = lambda: state.get("h")
        sys.modules["antenv.axon_hooks"] = mod
        import antenv
        antenv.axon_hooks = mod
        from trn_agent_boot.trn_boot import _ntff_profile_via_ctypes
        mod.set_axon_ntff_profile_hook(
            _ntff_profile_via_ctypes("/opt/axon/libaxon_pjrt.so"))
        return True
    except Exception as e:  # pragma: no cover
        print(f"NTFF hook shim failed: {e}")
        return False


def _run(inputs, trace=False):
    """Shard, execute on 8 cores, gather. Returns (y_full, exec_time_ns)."""
    from concourse.bass_utils import run_bass_kernel_spmd

    if trace:
        trace = _ensure_ntff_hook()

    wmap = _fold_weights(inputs)
    nc = _build_nc()

    x = np.asarray(inputs["x"], np.float32)
    pos = np.asarray(inputs["pos"], np.float32)
    w = int(np.asarray(inputs["w"]))
    order = _scanline_order(pos, w)
    x_ord = np.take_along_axis(x, order[..., None], axis=1)
    # device layout: [NTILES, 128 (token-in-sub), NSUB, DP]
    sw = np.zeros((NCORES, NTILES, 128, NSUB, DP), ml_dtypes.bfloat16)
    sw[..., 0:D] = x_ord.reshape(NCORES, NTILES, NSUB, 128, D).transpose(
        0, 1, 3, 2, 4)
    shards = sw.reshape(NCORES, NTILES, 128, NSUB * DP)

    wmap = {k: np.ascontiguousarray(v) for k, v in wmap.items()}
    in_maps = [{"x": shards[c], **wmap} for c in range(NCORES)]
    res = run_bass_kernel_spmd(nc, in_maps, core_ids=list(range(NCORES)),
                               trace=trace)
    y_ord = np.stack([np.asarray(res.results[c]["y"], np.float32)
                      for c in range(NCORES)])
    y_ord = y_ord.reshape(NCORES, NTILES, 128, NSUB, D).transpose(
        0, 1, 3, 2, 4)
    y_ord = y_ord.reshape(B, N, D)
    y = np.empty_like(y_ord)
    np.put_along_axis(y, order[..., None], y_ord, axis=1)
    return y.astype(np.float32), res.exec_time_ns


def kernel(**inputs):
    y, _ = _run(inputs, trace=False)
    return y


# revision 31
# speedup vs baseline: 1.1744x; 1.1744x over previous
"""Trainium2 Bass kernel for nn_BasicLayer (sparse cluster attention, 2 layers).

v6 (final): slot-scheduled pipeline + compact block-diagonal softmax + bf16
residual stream.  854,989 ns (staged baseline) -> 750,282 ns measured.

Host side: scanline (boustrophedon) gather into cluster order, data-parallel
over 8 cores (8192 tokens each, 16 supertiles of 512), LN affine + all biases
folded into bf16 matmul weights shipped as two packed blobs (biases are zero
for this problem; asserted).  x/y travel as bf16 (halves HBM traffic; rel err
3.7e-3 vs the 2e-2 gate).

On-device vs the staged v2 baseline:
- Scores are computed block-diagonally: two matmuls per head (one per
  64-token cluster, output partition ranges 0:64/64:128, col-tiled), each
  head in its own PSUM bank slot so the four concurrent row-group matmuls
  never share a bank (sharing one hangs the device).  This makes every
  softmax element valid: exp runs as two full-width ACT calls into a compact
  E[128,2,6,64], one full-width DVE reduce per sub, bf16 reciprocal, and the
  P multiply splits DVE/GpSimd halves into persistent zero-initialized P
  buffers (off-diagonal stays zero) that DMA-transpose whole, so the O
  matmuls are unchanged.
- Residual stream x is bf16 (frees 4KB/partition SBUF, 4x DVE tier for the
  LN normalize); LN1 normalize runs on the Scalar engine (Identity with
  per-partition scale/bias - identity is resident in every ACT table set so
  it costs no table switches), LN2 normalize on DVE; the fast-inverse-sqrt
  chain's tensor_tensor steps run on GpSimd.
- Emission is slot-scheduled: super-units (layer, half-of-tiles) with phases
  LN1/QKV/CH/PROJ+LN2/MLP at offsets 5*layer + 4*half, so DVE-heavy phases
  (LN, softmax) share slots with PE-heavy phases (MLP, QKV) of other units,
  thunk lists round-robin-interleaved within each slot.  This keeps all
  engines co-active, batches exp vs gelu ACT-table usage (~10 table loads vs
  50 for a naive interleave), and keeps the PE dense enough to limit HAM
  half-clock throttling.
- oPp has its own PSUM pool: sharing the "med" PSUM tag between the O
  matmuls and qkv/proj/fc produced a cross-pool slot cycle (phase_a waits on
  a v_tm slot freed by attn_out which waits on a PSUM slot held by phase_a)
  that Tile's scheduler simulation flags as a deadlock.
"""

import os
import numpy as np
import ml_dtypes

# ---- problem constants (hardcoded per contract) ----
B, N, D = 4, 16384, 192
DP = 256
HEADS, DH, CLM = 6, 32, 64
GRID_W = 128
DEPTH = 2
NCORES = 8
T = (B * N) // NCORES                # 8192 tokens per core
SUB = 128
NSUB = 4
TILE = SUB * NSUB                    # 512-token supertile
NTILES = T // TILE                   # 16
GPB = 4                              # tiles per pipeline group
DFF = 768

# packed weight blob column layout (per layer)
W0_COLS = 384 + 192 + 192 + 768 + 1152   # 2688
W1_COLS = 384 + 192 + 192 + 768          # 1536

_COMPILED = {}


def _scanline_order(pos, w):
    ix = np.floor(pos[..., 0]).astype(np.int64)
    iy = np.floor(pos[..., 1]).astype(np.int64)
    key = iy * w + np.where(iy % 2 == 1, w - 1 - ix, ix)
    return np.argsort(key, axis=1, kind="stable")


def _fold_weights(inputs):
    """Fold LN affine + biases into matmul weights; pack into two blobs."""
    bf16 = ml_dtypes.bfloat16
    scale = DH ** -0.5
    wb0 = np.zeros((128, DEPTH * W0_COLS), np.float64)
    wb1 = np.zeros((64, DEPTH * W1_COLS), np.float64)
    bias_norm = 0.0
    for i in range(DEPTH):
        g1 = np.asarray(inputs["ln1_g"][i], np.float64)
        b1 = np.asarray(inputs["ln1_b"][i], np.float64)
        Wqkv = np.asarray(inputs["w_qkv"][i], np.float64)
        bqkv = np.asarray(inputs["b_qkv"][i], np.float64)
        w_eff = g1[:, None] * Wqkv
        b_eff = b1 @ Wqkv + bqkv
        wq = w_eff[:, 0:D] * scale
        wk = w_eff[:, D:2 * D]
        wv = w_eff[:, 2 * D:3 * D]
        wqk = np.concatenate(
            [wq[:, :128], wk[:, :128], wq[:, 128:], wk[:, 128:]], axis=1)
        wp = np.asarray(inputs["w_proj"][i], np.float64)
        bp = np.asarray(inputs["b_proj"][i], np.float64)
        g2 = np.asarray(inputs["ln2_g"][i], np.float64)
        b2 = np.asarray(inputs["ln2_b"][i], np.float64)
        W1 = np.asarray(inputs["w_fc1"][i], np.float64)
        w1_eff = g2[:, None] * W1
        b1_eff = b2 @ W1 + np.asarray(inputs["b_fc1"][i], np.float64)
        W2 = np.asarray(inputs["w_fc2"][i], np.float64)
        bfc2 = np.asarray(inputs["b_fc2"][i], np.float64)
        bias_norm += (np.abs(b_eff).sum() + np.abs(bp).sum()
                      + np.abs(b1_eff).sum() + np.abs(bfc2).sum())
        w2m = W2.reshape(6, 128, D).transpose(1, 0, 2).reshape(128, 6 * D)
        c0 = i * W0_COLS
        wb0[:, c0:c0 + 384] = wqk[0:128]
        wb0[:, c0 + 384:c0 + 576] = wv[0:128]
        wb0[:, c0 + 576:c0 + 768] = wp[0:128]
        wb0[:, c0 + 768:c0 + 1536] = w1_eff[0:128]
        wb0[:, c0 + 1536:c0 + 2688] = w2m
        c1 = i * W1_COLS
        wb1[:, c1:c1 + 384] = wqk[128:192]
        wb1[:, c1 + 384:c1 + 576] = wv[128:192]
        wb1[:, c1 + 576:c1 + 768] = wp[128:192]
        wb1[:, c1 + 768:c1 + 1536] = w1_eff[128:192]
    if bias_norm > 1e-12:
        raise NotImplementedError(
            "kernel v3 assumes all folded biases are zero "
            "(true for this problem's setup_inputs)")
    return {"wb0": wb0.astype(bf16), "wb1": wb1.astype(bf16)}


def _build_nc(ntiles=NTILES):
    key = ("nc", ntiles, os.environ.get("K_NO_GPS"), os.environ.get("K_RECIP_F32"), os.environ.get("K_SEQ"), os.environ.get("K_BASE_SOFT"))
    if key in _COMPILED:
        return _COMPILED[key]

    from contextlib import ExitStack
    import concourse.bass as bass
    import concourse.tile as tile
    from concourse import bacc, mybir
    from concourse.bass import ts, ds

    f32 = mybir.dt.float32
    bf16 = mybir.dt.bfloat16
    i32 = mybir.dt.int32
    AF = mybir.ActivationFunctionType
    OP = mybir.AluOpType
    AX = mybir.AxisListType

    nc = bacc.Bacc("TRN2", target_bir_lowering=False, debug=False,
                   enable_asserts=False, num_devices=NCORES)

    x_d = nc.dram_tensor("x", [ntiles, 128, NSUB * DP], bf16,
                         kind="ExternalInput").ap()
    y_d = nc.dram_tensor("y", [ntiles, 128, NSUB * D], bf16,
                         kind="ExternalOutput").ap()
    wb0_d = nc.dram_tensor("wb0", [128, DEPTH * W0_COLS], bf16,
                           kind="ExternalInput").ap()
    wb1_d = nc.dram_tensor("wb1", [64, DEPTH * W1_COLS], bf16,
                           kind="ExternalInput").ap()

    with tile.TileContext(nc) as tc, ExitStack() as ctx:
        consts = ctx.enter_context(tc.tile_pool(name="consts", bufs=1))
        xpool = ctx.enter_context(tc.tile_pool(name="xpool", bufs=16))
        lnpool = ctx.enter_context(tc.tile_pool(name="lnpool", bufs=6))
        fmpool = ctx.enter_context(tc.tile_pool(name="fmpool", bufs=12))
        qkpool = ctx.enter_context(tc.tile_pool(name="qkpool", bufs=9))
        epool = ctx.enter_context(tc.tile_pool(name="epool", bufs=6))
        ofpool = ctx.enter_context(tc.tile_pool(name="ofpool", bufs=10))
        hpool = ctx.enter_context(tc.tile_pool(name="hpool", bufs=2))
        stpool = ctx.enter_context(tc.tile_pool(name="stpool", bufs=10))
        ppsc = ctx.enter_context(tc.tile_pool(name="ppsc", bufs=1, space="PSUM"))
        ppm = ctx.enter_context(tc.tile_pool(name="ppm", bufs=3, space="PSUM"))
        ppo = ctx.enter_context(tc.tile_pool(name="ppo", bufs=1, space="PSUM"))

        # persistent zeroed P buffers: off-diagonal blocks stay 0 forever
        NPK = 4
        pk_bufs = []
        for pb_i in range(NPK):
            pb = consts.tile([128, 2, HEADS, 128], bf16, name=f"pkbuf{pb_i}")
            nc.vector.memset(pb, 0.0)
            pk_bufs.append(pb)
        pk_ctr = [0]
        pkpool = ctx.enter_context(tc.tile_pool(name="pkpool", bufs=3))

        # --- packed weights, two DMAs ---
        wb0_t = consts.tile([128, DEPTH * W0_COLS], bf16, name="wb0")
        wb1_t = consts.tile([64, DEPTH * W1_COLS], bf16, name="wb1")
        nc.scalar.dma_start(out=wb0_t, in_=wb0_d)
        nc.scalar.dma_start(out=wb1_t, in_=wb1_d)
        W = []
        for i in range(DEPTH):
            c0 = i * W0_COLS
            c1 = i * W1_COLS
            W.append({
                "wqk0": wb0_t[:, c0:c0 + 384],
                "wv0": wb0_t[:, c0 + 384:c0 + 576],
                "wp0": wb0_t[:, c0 + 576:c0 + 768],
                "w10": wb0_t[:, c0 + 768:c0 + 1536],
                "w2m": wb0_t[:, c0 + 1536:c0 + 2688].rearrange(
                    "p (m n) -> p m n", m=6),
                "wqk1": wb1_t[:, c1:c1 + 384],
                "wv1": wb1_t[:, c1 + 384:c1 + 576],
                "wp1": wb1_t[:, c1 + 576:c1 + 768],
                "w11": wb1_t[:, c1 + 768:c1 + 1536],
            })

        MAGIC = 0x5F3759DF
        # CoreSim lacks Gelu_apprx_tanh; substitute Tanh for sim-only runs.
        GELU_FUNC = (AF.Tanh if os.environ.get("K_SIM_GELU_TANH") == "1"
                     else AF.Gelu_apprx_tanh)

        def load_x(it):
            x_t = xpool.tile([128, NSUB, DP], bf16, tag="x", name=f"x{it}")
            nc.sync.dma_start(
                out=x_t,
                in_=x_d[it].rearrange("p (s f) -> p s f", s=NSUB))
            return x_t

        def layernorm_fm(x_t, norm_on_act):
            """LN on token-major x_t -> feature-major bf16 via DMA transpose."""
            mv = stpool.tile([128, NSUB, 6], f32, tag="mv", name="mv")
            mv2 = stpool.tile([128, NSUB, 2], f32, tag="mv2", name="mv2")
            for s in range(NSUB):
                nc.vector.bn_stats(mv[:, s], x_t[:, s, 0:D])
                nc.vector.bn_aggr(mv2[:, s], mv[:, s])
            mean = mv2[:, :, 0]                      # [128, 4] stride 2
            var = mv2[:, :, 1]
            t_i = stpool.tile([128, NSUB], i32, tag="ti", name="t_i")
            y0 = stpool.tile([128, NSUB], f32, tag="y0", name="y0")
            zz = stpool.tile([128, NSUB], f32, tag="zz", name="zz")
            r4 = stpool.tile([128, NSUB], f32, tag="r4", name="r4")
            g = nc.vector if os.environ.get('K_NO_GPS') == '1' else nc.gpsimd
            nc.vector.tensor_scalar(
                out=t_i, in0=var.bitcast(i32), scalar1=1, scalar2=None,
                op0=OP.logical_shift_right)
            nc.vector.tensor_scalar(
                out=y0.bitcast(i32), in0=t_i, scalar1=MAGIC, scalar2=-1,
                op0=OP.subtract, op1=OP.mult)
            nc.vector.scalar_tensor_tensor(
                out=zz, in0=var, scalar=1e-5, in1=y0,
                op0=OP.add, op1=OP.mult)              # (var+eps)*y0
            g.tensor_tensor(out=zz, in0=zz, in1=y0, op=OP.mult)
            nc.vector.tensor_scalar(
                out=zz, in0=zz, scalar1=-0.5, scalar2=1.5,
                op0=OP.mult, op1=OP.add)              # 1.5 - 0.5 v y0^2
            g.tensor_tensor(out=r4, in0=zz, in1=y0, op=OP.mult)

            xn = lnpool.tile([128, 2, NSUB, 128], bf16, tag="xn", name="xn")
            if norm_on_act:
                mr = stpool.tile([128, NSUB], f32, tag="mr", name="mr")
                nmr = stpool.tile([128, NSUB], f32, tag="nmr", name="nmr")
                g.tensor_tensor(out=mr, in0=mean, in1=r4, op=OP.mult)
                nc.vector.tensor_scalar(
                    out=nmr, in0=mr, scalar1=-1.0, scalar2=None,
                    op0=OP.mult)                      # -mean*r
                for s in range(NSUB):
                    nc.scalar.activation(
                        xn[:, :, s],
                        x_t[:, s].rearrange("p (c f) -> p c f", c=2),
                        AF.Identity,
                        bias=nmr[:, s:s + 1], scale=r4[:, s:s + 1])
            else:
                for s in range(NSUB):
                    nc.vector.tensor_scalar(
                        out=xn[:, :, s],
                        in0=x_t[:, s].rearrange("p (c f) -> p c f", c=2),
                        scalar1=mean[:, s:s + 1], scalar2=r4[:, s:s + 1],
                        op0=OP.subtract, op1=OP.mult)
            fm2 = fmpool.tile([128, 2, NSUB, 128], bf16, tag="fm", name="fm")
            nc.sync.dma_start_transpose(out=fm2, in_=xn)
            return fm2[:, 0], fm2[:, 1]

        def phase_a(sb, fmA, fmA2):
            """qkv + v from feature-major LN output. Returns (qkA, qkB, v_tm)."""
            fmAf = fmA.rearrange("p a b -> p (a b)")
            fmA2f = fmA2.rearrange("p a b -> p (a b)")
            qkA = qkpool.tile([128, 2, TILE], bf16, tag="qkA", name="qkA")
            qkB = qkpool.tile([64, 2, TILE], bf16, tag="qkB", name="qkB")
            for m in range(2):
                ps = ppm.tile([128, TILE], f32, tag="med", name=f"psqA{m}")
                nc.tensor.matmul(ps, sb["wqk0"][:, ts(m, 128)], fmAf,
                                 start=True, stop=False)
                nc.tensor.matmul(ps, sb["wqk1"][:, ts(m, 128)],
                                 fmA2f[0:64], start=False, stop=True)
                nc.scalar.activation(qkA[:, m], ps, AF.Copy)
            for m in range(2):
                ps = ppm.tile([64, TILE], f32, tag="med", name=f"psqB{m}")
                nc.tensor.matmul(ps, sb["wqk0"][:, ds(256 + m * 64, 64)],
                                 fmAf, start=True, stop=False)
                nc.tensor.matmul(ps, sb["wqk1"][:, ds(256 + m * 64, 64)],
                                 fmA2f[0:64], start=False, stop=True)
                nc.scalar.activation(qkB[:, m], ps, AF.Copy)
            v_tm = qkpool.tile([128, NSUB, D], bf16, tag="vtm", name="v_tm")
            for sp in range(2):
                psv = ppm.tile([128, 2, 256], f32, tag="med", name="psv")
                for j in range(2):
                    s = sp * 2 + j
                    nc.tensor.matmul(psv[:, j, 0:D], fmA[:, s], sb["wv0"],
                                     start=True, stop=False)
                    nc.tensor.matmul(psv[:, j, 0:D], fmA2[0:64, s], sb["wv1"],
                                     start=False, stop=True)
                nc.scalar.activation(v_tm[:, ds(sp * 2, 2)],
                                     psv[:, :, 0:D], AF.Copy)
            return qkA, qkB, v_tm

        def attn_soft(qkA, qkB, sp):
            """Compact block-diagonal scores + softmax + P^T for one sub-pair.

            Scores are computed with two matmuls per head (one per 64-token
            cluster, output partition ranges 0:64 / 64:128) into a compact
            all-valid [128, h, 64] layout, so exp / reduce / recip run at
            full partition width with no wasted work.  Head h lives at
            (m=h%4, 64-col slot h//4) so the four concurrently-running
            row-group matmuls land in different PSUM banks.
            """
            E2 = epool.tile([128, 2, HEADS, 64], bf16, tag="E2", name="E2")
            P2 = pk_bufs[pk_ctr[0] % NPK]
            pk_ctr[0] += 1
            for j in range(2):
                s = sp * 2 + j
                sc = ppsc.tile([128, 4, 512], f32, tag="sc", name="sc")
                scv = sc.rearrange("p m (g f) -> p g m f", g=8)
                for h in range(HEADS):
                    if h < 4:
                        qrow, kq = ts(h, 32), qkA
                        mslot, gslot = h, 0
                    else:
                        qrow, kq = ts(h - 4, 32), qkB
                        mslot, gslot = h - 4, 1
                    for c in range(2):
                        ccols = ds(s * 128 + c * 64, 64)
                        nc.tensor.matmul(
                            sc[ds(c * 64, 64), mslot, ds(gslot * 64, 64)],
                            kq[qrow, 0, ccols], kq[qrow, 1, ccols],
                            start=True, stop=True,
                            tile_position=(32 * mslot, c * 64))
                nc.scalar.activation(E2[:, j, 0:4], scv[:, 0], AF.Exp)
                nc.scalar.activation(E2[:, j, 4:6], scv[:, 1, 0:2], AF.Exp)
                sums = stpool.tile([128, HEADS], f32, tag="sm", name="sums")
                rsum = stpool.tile([128, HEADS], bf16, tag="rs", name="rsum")
                nc.vector.reduce_sum(sums, E2[:, j], axis=AX.X)
                with nc.allow_low_precision(
                        reason="bf16 softmax reciprocal is well within "
                               "the 2e-2 relative error budget"):
                    nc.vector.reciprocal(rsum, sums)
                for half in range(2):
                    hs = ds(half * 64, 64)
                    rs_half = rsum[hs]
                    rsum_b = bass.AP(tensor=rs_half.tensor,
                                     offset=rs_half.offset,
                                     ap=[*rs_half.ap, [0, 64]])
                    eng = nc.vector if half == 0 else nc.gpsimd
                    eng.tensor_tensor(
                        out=P2[hs, j, :, ds(half * 64, 64)],
                        in0=E2[hs, j], in1=rsum_b, op=OP.mult)
            pkm2 = pkpool.tile([128, 2, HEADS, 128], bf16, tag="pkm",
                               name="pkm")
            nc.sync.dma_start_transpose(out=pkm2, in_=P2)
            return pkm2

        def attn_out(v_tm, pk2, ofmA, ofmB, sp):
            """attention O matmuls + feature-major output copies."""
            oPp = ppo.tile([128, 2, 256], f32, tag="oPp", name="oPp")
            for j in range(2):
                s = sp * 2 + j
                for h in range(HEADS):
                    if h < 4:
                        out = oPp[ts(h, 32), j, 0:128]
                        colpos = h * 32
                    else:
                        out = oPp[ts(h - 4, 32), j, 128:256]
                        colpos = (h - 4) * 32
                    nc.tensor.matmul(out, v_tm[:, s, ts(h, 32)],
                                     pk2[:, j, h], start=True, stop=True,
                                     tile_position=(0, colpos))
            nc.vector.tensor_copy(
                ofmA.rearrange("p (a b) -> p a b", a=NSUB)[:, ds(sp * 2, 2)],
                oPp[:, :, 0:128])
            nc.vector.tensor_copy(
                ofmB.rearrange("p (a b) -> p a b", a=NSUB)[:, ds(sp * 2, 2)],
                oPp[0:64, :, 128:256])

        def phase_c_proj(sb, x_t, ofmA, ofmB):
            for sp in range(2):
                psp = ppm.tile([128, 2, 256], f32, tag="med", name="psp")
                for j in range(2):
                    s = sp * 2 + j
                    nc.tensor.matmul(psp[:, j, 0:D], ofmA[:, ts(s, 128)],
                                     sb["wp0"], start=True, stop=False)
                    nc.tensor.matmul(psp[:, j, 0:D], ofmB[:, ts(s, 128)],
                                     sb["wp1"], start=False, stop=True)
                nc.vector.tensor_add(x_t[:, ds(sp * 2, 2), 0:D],
                                     x_t[:, ds(sp * 2, 2), 0:D],
                                     psp[:, :, 0:D])

        def phase_c_mlp(sb, x_t, ynA, ynA2):
            ynAf = ynA.rearrange("p a b -> p (a b)")
            ynA2f = ynA2.rearrange("p a b -> p (a b)")
            hfm = hpool.tile([128, 6, TILE], bf16, tag="hfm", name="hfm")
            for m in range(6):
                psf = ppm.tile([128, TILE], f32, tag="med", name="psf1")
                nc.tensor.matmul(psf, sb["w10"][:, ts(m, 128)],
                                 ynAf, start=True, stop=False)
                nc.tensor.matmul(psf, sb["w11"][:, ts(m, 128)],
                                 ynA2f[0:64], start=False, stop=True)
                nc.scalar.activation(hfm[:, m], psf, GELU_FUNC)
            for sp in range(2):
                psf2 = ppm.tile([128, 2, 256], f32, tag="med", name="psf2")
                for j in range(2):
                    s = sp * 2 + j
                    for m in range(6):
                        nc.tensor.matmul(psf2[:, j, 0:D],
                                         hfm[:, m, ts(s, 128)],
                                         sb["w2m"][:, m],
                                         start=(m == 0), stop=(m == 5))
                nc.vector.tensor_add(x_t[:, ds(sp * 2, 2), 0:D],
                                     x_t[:, ds(sp * 2, 2), 0:D],
                                     psf2[:, :, 0:D])

        def store_y(it, x_t):
            nc.sync.dma_start(
                out=y_d[it].rearrange("p (s f) -> p s f", s=NSUB),
                in_=x_t[:, :, 0:D])

        # ---- slot-scheduled emission (v4 structure) ----
        # Five phases per super-unit (layer, half-of-tiles):
        # LN1 / QKV / CH / PROJ+LN2 / MLP, with unit offsets chosen so
        # DVE-heavy phases share slots with PE-heavy phases of other units
        # and exp/gelu stay batched per slot.
        GPB2 = min(8, ntiles)
        nstream = ntiles // GPB2
        US = []
        for li in range(DEPTH):
            for s in range(nstream):
                US.append((li, [s * GPB2 + t for t in range(GPB2)]))
        offs = []
        for ui, (li, _) in enumerate(US):
            s = ui % nstream if nstream > 1 else 0
            offs.append(5 * li + 4 * s)

        xts = {}
        fms = {}
        qks = {}
        ofs_ = {}
        yns = {}

        def ln1_thunks(u):
            li, tiles = u
            out = []
            for it in tiles:
                def th(li=li, it=it):
                    fms[(li, it)] = layernorm_fm(xts[it], norm_on_act=True)
                out.append(th)
            return out

        def qkv_thunks(u):
            li, tiles = u
            out = []
            for it in tiles:
                def th(li=li, it=it):
                    qks[(li, it)] = phase_a(W[li], *fms.pop((li, it)))
                out.append(th)
            return out

        def ch_thunks(u):
            li, tiles = u
            thunks = []
            for it in tiles:
                def alloc(li=li, it=it):
                    ofs_[(li, it)] = (
                        ofpool.tile([128, TILE], bf16, tag="ofA",
                                    name="ofmA"),
                        ofpool.tile([64, TILE], bf16, tag="ofB",
                                    name="ofmB"))
                thunks.append(alloc)
            chains = [(it, sp) for it in tiles for sp in range(2)]
            SKEW = 2
            pks = {}
            for ci in range(len(chains) + SKEW):
                if ci < len(chains):
                    def soft(ci=ci, li=li):
                        it, sp = chains[ci]
                        qkA, qkB, _ = qks[(li, it)]
                        pks[ci] = attn_soft(qkA, qkB, sp)
                    thunks.append(soft)
                if ci >= SKEW:
                    def outt(ci=ci, li=li):
                        it, sp = chains[ci - SKEW]
                        _, _, v_tm = qks[(li, it)]
                        attn_out(v_tm, pks.pop(ci - SKEW),
                                 ofs_[(li, it)][0], ofs_[(li, it)][1], sp)
                    thunks.append(outt)
            def drop(li=li, tiles=tiles):
                for it in tiles:
                    qks.pop((li, it))
            thunks.append(drop)
            return thunks

        def projln2_thunks(u):
            li, tiles = u
            n = len(tiles)
            thunks = []
            for i in range(n + 1):
                if i < n:
                    it = tiles[i]
                    def proj(li=li, it=it):
                        ofmA, ofmB = ofs_.pop((li, it))
                        phase_c_proj(W[li], xts[it], ofmA, ofmB)
                    thunks.append(proj)
                if 0 <= i - 1 < n:
                    it2 = tiles[i - 1]
                    def ln2(li=li, it=it2):
                        yns[(li, it)] = layernorm_fm(xts[it],
                                                     norm_on_act=False)
                    thunks.append(ln2)
            return thunks

        def mlp_thunks(u):
            li, tiles = u
            out = []
            for it in tiles:
                def th(li=li, it=it):
                    phase_c_mlp(W[li], xts[it], *yns.pop((li, it)))
                    if li == DEPTH - 1:
                        store_y(it, xts[it])
                out.append(th)
            return out

        PHASES = [ln1_thunks, qkv_thunks, ch_thunks, projln2_thunks,
                  mlp_thunks]

        from collections import defaultdict as _dd
        slots = _dd(list)
        for ui, u in enumerate(US):
            for p in range(5):
                slots[offs[ui] + p].append((p, u))

        for it in range(ntiles):
            xts[it] = load_x(it)

        def interleave(lists):
            idx = [0] * len(lists)
            total = sum(len(l) for l in lists)
            for _ in range(total):
                best, bf = None, 2.0
                for li_, l in enumerate(lists):
                    if idx[li_] < len(l):
                        f = idx[li_] / len(l)
                        if f < bf:
                            best, bf = li_, f
                lists[best][idx[best]]()
                idx[best] += 1

        for sl in sorted(slots):
            entries = slots[sl]
            entries.sort(key=lambda e: (e[0] != 2, e[0]))
            interleave([PHASES[p](u) for p, u in entries])

    nc.compile()
    _COMPILED[key] = nc
    return nc


def _ensure_ntff_hook():
    import sys, types
    if "antenv.axon_hooks" in sys.modules:
        return True
    try:
        mod = types.ModuleType("antenv.axon_hooks")
        state = {}
        mod.set_axon_ntff_profile_hook = lambda h: state.__setitem__("h", h)
        mod.get_axon_ntff_profile_hook = lambda: state.get("h")
        sys.modules["antenv.axon_hooks"] = mod
        import antenv
        antenv.axon_hooks = mod
        from trn_agent_boot.trn_boot import _ntff_profile_via_ctypes
        mod.set_axon_ntff_profile_hook(
            _ntff_profile_via_ctypes("/opt/axon/libaxon_pjrt.so"))
        return True
    except Exception as e:  # pragma: no cover
        print(f"NTFF hook shim failed: {e}")
        return False


def _run(inputs, trace=False):
    """Shard, execute on 8 cores, gather. Returns (y_full, exec_time_ns)."""
    from concourse.bass_utils import run_bass_kernel_spmd

    if trace:
        trace = _ensure_ntff_hook()

    wmap = _fold_weights(inputs)
    nc = _build_nc()

    x = np.asarray(inputs["x"], np.float32)
    pos = np.asarray(inputs["pos"], np.float32)
    w = int(np.asarray(inputs["w"]))
    order = _scanline_order(pos, w)
    x_ord = np.take_along_axis(x, order[..., None], axis=1)
    # device layout: [NTILES, 128 (token-in-sub), NSUB, DP]
    sw = np.zeros((NCORES, NTILES, 128, NSUB, DP), ml_dtypes.bfloat16)
    sw[..., 0:D] = x_ord.reshape(NCORES, NTILES, NSUB, 128, D).transpose(
        0, 1, 3, 2, 4)
    shards = sw.reshape(NCORES, NTILES, 128, NSUB * DP)

    wmap = {k: np.ascontiguousarray(v) for k, v in wmap.items()}
    in_maps = [{"x": shards[c], **wmap} for c in range(NCORES)]
    res = run_bass_kernel_spmd(nc, in_maps, core_ids=list(range(NCORES)),
                               trace=trace)
    y_ord = np.stack([np.asarray(res.results[c]["y"], np.float32)
                      for c in range(NCORES)])
    y_ord = y_ord.reshape(NCORES, NTILES, 128, NSUB, D).transpose(
        0, 1, 3, 2, 4)
    y_ord = y_ord.reshape(B, N, D)
    y = np.empty_like(y_ord)
    np.put_along_axis(y, order[..., None], y_ord, axis=1)
    return y.astype(np.float32), res.exec_time_ns


def kernel(**inputs):
    y, _ = _run(inputs, trace=False)
    return y


# revision 32
# speedup vs baseline: 1.1998x; 1.0217x over previous
"""Trainium2 Bass kernel for nn_BasicLayer (sparse cluster attention, 2 layers).

v6 (final): slot-scheduled pipeline + compact block-diagonal softmax + bf16
residual stream.  854,989 ns (staged baseline) -> 750,282 ns measured.

Host side: scanline (boustrophedon) gather into cluster order, data-parallel
over 8 cores (8192 tokens each, 16 supertiles of 512), LN affine + all biases
folded into bf16 matmul weights shipped as two packed blobs (biases are zero
for this problem; asserted).  x/y travel as bf16 (halves HBM traffic; rel err
3.7e-3 vs the 2e-2 gate).

On-device vs the staged v2 baseline:
- Scores are computed block-diagonally: two matmuls per head (one per
  64-token cluster, output partition ranges 0:64/64:128, col-tiled), each
  head in its own PSUM bank slot so the four concurrent row-group matmuls
  never share a bank (sharing one hangs the device).  This makes every
  softmax element valid: exp runs as two full-width ACT calls into a compact
  E[128,2,6,64], one full-width DVE reduce per sub, bf16 reciprocal, and the
  P multiply splits DVE/GpSimd halves into persistent zero-initialized P
  buffers (off-diagonal stays zero) that DMA-transpose whole, so the O
  matmuls are unchanged.
- Residual stream x is bf16 (frees 4KB/partition SBUF, 4x DVE tier for the
  LN normalize); LN1 normalize runs on the Scalar engine (Identity with
  per-partition scale/bias - identity is resident in every ACT table set so
  it costs no table switches), LN2 normalize on DVE; the fast-inverse-sqrt
  chain's tensor_tensor steps run on GpSimd.
- Emission is slot-scheduled: super-units (layer, half-of-tiles) with phases
  LN1/QKV/CH/PROJ+LN2/MLP at offsets 5*layer + 4*half, so DVE-heavy phases
  (LN, softmax) share slots with PE-heavy phases (MLP, QKV) of other units,
  thunk lists round-robin-interleaved within each slot.  This keeps all
  engines co-active, batches exp vs gelu ACT-table usage (~10 table loads vs
  50 for a naive interleave), and keeps the PE dense enough to limit HAM
  half-clock throttling.
- oPp has its own PSUM pool: sharing the "med" PSUM tag between the O
  matmuls and qkv/proj/fc produced a cross-pool slot cycle (phase_a waits on
  a v_tm slot freed by attn_out which waits on a PSUM slot held by phase_a)
  that Tile's scheduler simulation flags as a deadlock.
"""

import os
import numpy as np
import ml_dtypes

# ---- problem constants (hardcoded per contract) ----
B, N, D = 4, 16384, 192
DP = 256
HEADS, DH, CLM = 6, 32, 64
GRID_W = 128
DEPTH = 2
NCORES = 8
T = (B * N) // NCORES                # 8192 tokens per core
SUB = 128
NSUB = 4
TILE = SUB * NSUB                    # 512-token supertile
NTILES = T // TILE                   # 16
GPB = 4                              # tiles per pipeline group
DFF = 768

# packed weight blob column layout (per layer)
W0_COLS = 384 + 192 + 192 + 768 + 1152   # 2688
W1_COLS = 384 + 192 + 192 + 768          # 1536

_COMPILED = {}


def _scanline_order(pos, w):
    ix = np.floor(pos[..., 0]).astype(np.int64)
    iy = np.floor(pos[..., 1]).astype(np.int64)
    key = iy * w + np.where(iy % 2 == 1, w - 1 - ix, ix)
    return np.argsort(key, axis=1, kind="stable")


def _fold_weights(inputs):
    """Fold LN affine + biases into matmul weights; pack into two blobs."""
    bf16 = ml_dtypes.bfloat16
    scale = DH ** -0.5
    wb0 = np.zeros((128, DEPTH * W0_COLS), np.float64)
    wb1 = np.zeros((64, DEPTH * W1_COLS), np.float64)
    bias_norm = 0.0
    for i in range(DEPTH):
        g1 = np.asarray(inputs["ln1_g"][i], np.float64)
        b1 = np.asarray(inputs["ln1_b"][i], np.float64)
        Wqkv = np.asarray(inputs["w_qkv"][i], np.float64)
        bqkv = np.asarray(inputs["b_qkv"][i], np.float64)
        w_eff = g1[:, None] * Wqkv
        b_eff = b1 @ Wqkv + bqkv
        wq = w_eff[:, 0:D] * scale
        wk = w_eff[:, D:2 * D]
        wv = w_eff[:, 2 * D:3 * D]
        wqk = np.concatenate(
            [wq[:, :128], wk[:, :128], wq[:, 128:], wk[:, 128:]], axis=1)
        wp = np.asarray(inputs["w_proj"][i], np.float64)
        bp = np.asarray(inputs["b_proj"][i], np.float64)
        g2 = np.asarray(inputs["ln2_g"][i], np.float64)
        b2 = np.asarray(inputs["ln2_b"][i], np.float64)
        W1 = np.asarray(inputs["w_fc1"][i], np.float64)
        w1_eff = g2[:, None] * W1
        b1_eff = b2 @ W1 + np.asarray(inputs["b_fc1"][i], np.float64)
        W2 = np.asarray(inputs["w_fc2"][i], np.float64)
        bfc2 = np.asarray(inputs["b_fc2"][i], np.float64)
        bias_norm += (np.abs(b_eff).sum() + np.abs(bp).sum()
                      + np.abs(b1_eff).sum() + np.abs(bfc2).sum())
        w2m = W2.reshape(6, 128, D).transpose(1, 0, 2).reshape(128, 6 * D)
        c0 = i * W0_COLS
        wb0[:, c0:c0 + 384] = wqk[0:128]
        wb0[:, c0 + 384:c0 + 576] = wv[0:128]
        wb0[:, c0 + 576:c0 + 768] = wp[0:128]
        wb0[:, c0 + 768:c0 + 1536] = w1_eff[0:128]
        wb0[:, c0 + 1536:c0 + 2688] = w2m
        c1 = i * W1_COLS
        wb1[:, c1:c1 + 384] = wqk[128:192]
        wb1[:, c1 + 384:c1 + 576] = wv[128:192]
        wb1[:, c1 + 576:c1 + 768] = wp[128:192]
        wb1[:, c1 + 768:c1 + 1536] = w1_eff[128:192]
    if bias_norm > 1e-12:
        raise NotImplementedError(
            "kernel v3 assumes all folded biases are zero "
            "(true for this problem's setup_inputs)")
    return {"wb0": wb0.astype(bf16), "wb1": wb1.astype(bf16)}


def _build_nc(ntiles=NTILES):
    key = ("nc", ntiles, os.environ.get("K_NO_GPS"), os.environ.get("K_RECIP_F32"), os.environ.get("K_SEQ"), os.environ.get("K_BASE_SOFT"))
    if key in _COMPILED:
        return _COMPILED[key]

    from contextlib import ExitStack
    import concourse.bass as bass
    import concourse.tile as tile
    from concourse import bacc, mybir
    from concourse.bass import ts, ds

    f32 = mybir.dt.float32
    bf16 = mybir.dt.bfloat16
    i32 = mybir.dt.int32
    AF = mybir.ActivationFunctionType
    OP = mybir.AluOpType
    AX = mybir.AxisListType

    nc = bacc.Bacc("TRN2", target_bir_lowering=False, debug=False,
                   enable_asserts=False, num_devices=NCORES)

    x_d = nc.dram_tensor("x", [ntiles, 128, NSUB * DP], bf16,
                         kind="ExternalInput").ap()
    y_d = nc.dram_tensor("y", [ntiles, 128, NSUB * D], bf16,
                         kind="ExternalOutput").ap()
    wb0_d = nc.dram_tensor("wb0", [128, DEPTH * W0_COLS], bf16,
                           kind="ExternalInput").ap()
    wb1_d = nc.dram_tensor("wb1", [64, DEPTH * W1_COLS], bf16,
                           kind="ExternalInput").ap()

    with tile.TileContext(nc) as tc, ExitStack() as ctx:
        consts = ctx.enter_context(tc.tile_pool(name="consts", bufs=1))
        xpool = ctx.enter_context(tc.tile_pool(name="xpool", bufs=16))
        lnpool = ctx.enter_context(tc.tile_pool(name="lnpool", bufs=6))
        fmpool = ctx.enter_context(tc.tile_pool(name="fmpool", bufs=12))
        qkpool = ctx.enter_context(tc.tile_pool(name="qkpool", bufs=9))
        epool = ctx.enter_context(tc.tile_pool(name="epool", bufs=6))
        ofpool = ctx.enter_context(tc.tile_pool(name="ofpool", bufs=10))
        hpool = ctx.enter_context(tc.tile_pool(name="hpool", bufs=2))
        stpool = ctx.enter_context(tc.tile_pool(name="stpool", bufs=10))
        ppsc = ctx.enter_context(tc.tile_pool(name="ppsc", bufs=1, space="PSUM"))
        ppm = ctx.enter_context(tc.tile_pool(name="ppm", bufs=3, space="PSUM"))
        ppo = ctx.enter_context(tc.tile_pool(name="ppo", bufs=1, space="PSUM"))

        # persistent zeroed P buffers: off-diagonal blocks stay 0 forever
        NPK = 4
        pk_bufs = []
        for pb_i in range(NPK):
            pb = consts.tile([128, 2, HEADS, 128], bf16, name=f"pkbuf{pb_i}")
            nc.vector.memset(pb, 0.0)
            pk_bufs.append(pb)
        pk_ctr = [0]
        pkpool = ctx.enter_context(tc.tile_pool(name="pkpool", bufs=4))

        # --- packed weights, two DMAs ---
        wb0_t = consts.tile([128, DEPTH * W0_COLS], bf16, name="wb0")
        wb1_t = consts.tile([64, DEPTH * W1_COLS], bf16, name="wb1")
        nc.scalar.dma_start(out=wb0_t, in_=wb0_d)
        nc.scalar.dma_start(out=wb1_t, in_=wb1_d)
        W = []
        for i in range(DEPTH):
            c0 = i * W0_COLS
            c1 = i * W1_COLS
            W.append({
                "wqk0": wb0_t[:, c0:c0 + 384],
                "wv0": wb0_t[:, c0 + 384:c0 + 576],
                "wp0": wb0_t[:, c0 + 576:c0 + 768],
                "w10": wb0_t[:, c0 + 768:c0 + 1536],
                "w2m": wb0_t[:, c0 + 1536:c0 + 2688].rearrange(
                    "p (m n) -> p m n", m=6),
                "wqk1": wb1_t[:, c1:c1 + 384],
                "wv1": wb1_t[:, c1 + 384:c1 + 576],
                "wp1": wb1_t[:, c1 + 576:c1 + 768],
                "w11": wb1_t[:, c1 + 768:c1 + 1536],
            })

        MAGIC = 0x5F3759DF
        # CoreSim lacks Gelu_apprx_tanh; substitute Tanh for sim-only runs.
        GELU_FUNC = (AF.Tanh if os.environ.get("K_SIM_GELU_TANH") == "1"
                     else AF.Gelu_apprx_tanh)

        def load_x(it):
            x_t = xpool.tile([128, NSUB, DP], bf16, tag="x", name=f"x{it}")
            nc.sync.dma_start(
                out=x_t,
                in_=x_d[it].rearrange("p (s f) -> p s f", s=NSUB))
            return x_t

        def layernorm_fm(x_t, norm_on_act):
            """LN on token-major x_t -> feature-major bf16 via DMA transpose."""
            mv = stpool.tile([128, NSUB, 6], f32, tag="mv", name="mv")
            mv2 = stpool.tile([128, NSUB, 2], f32, tag="mv2", name="mv2")
            for s in range(NSUB):
                nc.vector.bn_stats(mv[:, s], x_t[:, s, 0:D])
                nc.vector.bn_aggr(mv2[:, s], mv[:, s])
            mean = mv2[:, :, 0]                      # [128, 4] stride 2
            var = mv2[:, :, 1]
            t_i = stpool.tile([128, NSUB], i32, tag="ti", name="t_i")
            y0 = stpool.tile([128, NSUB], f32, tag="y0", name="y0")
            zz = stpool.tile([128, NSUB], f32, tag="zz", name="zz")
            r4 = stpool.tile([128, NSUB], f32, tag="r4", name="r4")
            g = nc.vector if os.environ.get('K_NO_GPS') == '1' else nc.gpsimd
            nc.vector.tensor_scalar(
                out=t_i, in0=var.bitcast(i32), scalar1=1, scalar2=None,
                op0=OP.logical_shift_right)
            nc.vector.tensor_scalar(
                out=y0.bitcast(i32), in0=t_i, scalar1=MAGIC, scalar2=-1,
                op0=OP.subtract, op1=OP.mult)
            nc.vector.scalar_tensor_tensor(
                out=zz, in0=var, scalar=1e-5, in1=y0,
                op0=OP.add, op1=OP.mult)              # (var+eps)*y0
            g.tensor_tensor(out=zz, in0=zz, in1=y0, op=OP.mult)
            nc.vector.tensor_scalar(
                out=zz, in0=zz, scalar1=-0.5, scalar2=1.5,
                op0=OP.mult, op1=OP.add)              # 1.5 - 0.5 v y0^2
            g.tensor_tensor(out=r4, in0=zz, in1=y0, op=OP.mult)

            xn = lnpool.tile([128, 2, NSUB, 128], bf16, tag="xn", name="xn")
            if norm_on_act:
                mr = stpool.tile([128, NSUB], f32, tag="mr", name="mr")
                nmr = stpool.tile([128, NSUB], f32, tag="nmr", name="nmr")
                g.tensor_tensor(out=mr, in0=mean, in1=r4, op=OP.mult)
                nc.vector.tensor_scalar(
                    out=nmr, in0=mr, scalar1=-1.0, scalar2=None,
                    op0=OP.mult)                      # -mean*r
                for s in range(NSUB):
                    nc.scalar.activation(
                        xn[:, :, s],
                        x_t[:, s].rearrange("p (c f) -> p c f", c=2),
                        AF.Identity,
                        bias=nmr[:, s:s + 1], scale=r4[:, s:s + 1])
            else:
                for s in range(NSUB):
                    nc.vector.tensor_scalar(
                        out=xn[:, :, s],
                        in0=x_t[:, s].rearrange("p (c f) -> p c f", c=2),
                        scalar1=mean[:, s:s + 1], scalar2=r4[:, s:s + 1],
                        op0=OP.subtract, op1=OP.mult)
            fm2 = fmpool.tile([128, 2, NSUB, 128], bf16, tag="fm", name="fm")
            nc.sync.dma_start_transpose(out=fm2, in_=xn)
            return fm2[:, 0], fm2[:, 1]

        def phase_a(sb, fmA, fmA2):
            """qkv + v from feature-major LN output. Returns (qkA, qkB, v_tm)."""
            fmAf = fmA.rearrange("p a b -> p (a b)")
            fmA2f = fmA2.rearrange("p a b -> p (a b)")
            qkA = qkpool.tile([128, 2, TILE], bf16, tag="qkA", name="qkA")
            qkB = qkpool.tile([64, 2, TILE], bf16, tag="qkB", name="qkB")
            for m in range(2):
                ps = ppm.tile([128, TILE], f32, tag="med", name=f"psqA{m}")
                nc.tensor.matmul(ps, sb["wqk0"][:, ts(m, 128)], fmAf,
                                 start=True, stop=False)
                nc.tensor.matmul(ps, sb["wqk1"][:, ts(m, 128)],
                                 fmA2f[0:64], start=False, stop=True)
                nc.scalar.activation(qkA[:, m], ps, AF.Copy)
            for m in range(2):
                ps = ppm.tile([64, TILE], f32, tag="med", name=f"psqB{m}")
                nc.tensor.matmul(ps, sb["wqk0"][:, ds(256 + m * 64, 64)],
                                 fmAf, start=True, stop=False)
                nc.tensor.matmul(ps, sb["wqk1"][:, ds(256 + m * 64, 64)],
                                 fmA2f[0:64], start=False, stop=True)
                nc.scalar.activation(qkB[:, m], ps, AF.Copy)
            v_tm = qkpool.tile([128, NSUB, D], bf16, tag="vtm", name="v_tm")
            for sp in range(2):
                psv = ppm.tile([128, 2, 256], f32, tag="med", name="psv")
                for j in range(2):
                    s = sp * 2 + j
                    nc.tensor.matmul(psv[:, j, 0:D], fmA[:, s], sb["wv0"],
                                     start=True, stop=False)
                    nc.tensor.matmul(psv[:, j, 0:D], fmA2[0:64, s], sb["wv1"],
                                     start=False, stop=True)
                nc.scalar.activation(v_tm[:, ds(sp * 2, 2)],
                                     psv[:, :, 0:D], AF.Copy)
            return qkA, qkB, v_tm

        def attn_soft(qkA, qkB, sp):
            """Compact block-diagonal scores + softmax + P^T for one sub-pair.

            Scores are computed with two matmuls per head (one per 64-token
            cluster, output partition ranges 0:64 / 64:128) into a compact
            all-valid [128, h, 64] layout, so exp / reduce / recip run at
            full partition width with no wasted work.  Head h lives at
            (m=h%4, 64-col slot h//4) so the four concurrently-running
            row-group matmuls land in different PSUM banks.
            """
            E2 = epool.tile([128, 2, HEADS, 64], bf16, tag="E2", name="E2")
            P2 = pk_bufs[pk_ctr[0] % NPK]
            pk_ctr[0] += 1
            for j in range(2):
                s = sp * 2 + j
                sc = ppsc.tile([128, 4, 512], f32, tag="sc", name="sc")
                scv = sc.rearrange("p m (g f) -> p g m f", g=8)
                for h in range(HEADS):
                    if h < 4:
                        qrow, kq = ts(h, 32), qkA
                        mslot, gslot = h, 0
                    else:
                        qrow, kq = ts(h - 4, 32), qkB
                        mslot, gslot = h - 4, 1
                    for c in range(2):
                        ccols = ds(s * 128 + c * 64, 64)
                        nc.tensor.matmul(
                            sc[ds(c * 64, 64), mslot, ds(gslot * 64, 64)],
                            kq[qrow, 0, ccols], kq[qrow, 1, ccols],
                            start=True, stop=True,
                            tile_position=(32 * mslot, c * 64))
                nc.scalar.activation(E2[:, j, 0:4], scv[:, 0], AF.Exp)
                nc.scalar.activation(E2[:, j, 4:6], scv[:, 1, 0:2], AF.Exp)
                sums = stpool.tile([128, HEADS], f32, tag="sm", name="sums")
                rsum = stpool.tile([128, HEADS], bf16, tag="rs", name="rsum")
                nc.vector.reduce_sum(sums, E2[:, j], axis=AX.X)
                with nc.allow_low_precision(
                        reason="bf16 softmax reciprocal is well within "
                               "the 2e-2 relative error budget"):
                    nc.vector.reciprocal(rsum, sums)
                for half in range(2):
                    hs = ds(half * 64, 64)
                    rs_half = rsum[hs]
                    rsum_b = bass.AP(tensor=rs_half.tensor,
                                     offset=rs_half.offset,
                                     ap=[*rs_half.ap, [0, 64]])
                    eng = nc.vector if half == 0 else nc.gpsimd
                    eng.tensor_tensor(
                        out=P2[hs, j, :, ds(half * 64, 64)],
                        in0=E2[hs, j], in1=rsum_b, op=OP.mult)
            pkm2 = pkpool.tile([128, 2, HEADS, 128], bf16, tag="pkm",
                               name="pkm")
            nc.sync.dma_start_transpose(out=pkm2, in_=P2)
            return pkm2

        def attn_out(v_tm, pk2, ofmA, ofmB, sp):
            """attention O matmuls + feature-major output copies."""
            oPp = ppo.tile([128, 2, 256], f32, tag="oPp", name="oPp")
            for j in range(2):
                s = sp * 2 + j
                for h in range(HEADS):
                    if h < 4:
                        out = oPp[ts(h, 32), j, 0:128]
                        colpos = h * 32
                    else:
                        out = oPp[ts(h - 4, 32), j, 128:256]
                        colpos = (h - 4) * 32
                    nc.tensor.matmul(out, v_tm[:, s, ts(h, 32)],
                                     pk2[:, j, h], start=True, stop=True,
                                     tile_position=(0, colpos))
            nc.vector.tensor_copy(
                ofmA.rearrange("p (a b) -> p a b", a=NSUB)[:, ds(sp * 2, 2)],
                oPp[:, :, 0:128])
            nc.vector.tensor_copy(
                ofmB.rearrange("p (a b) -> p a b", a=NSUB)[:, ds(sp * 2, 2)],
                oPp[0:64, :, 128:256])

        def phase_c_proj(sb, x_t, ofmA, ofmB):
            for sp in range(2):
                psp = ppm.tile([128, 2, 256], f32, tag="med", name="psp")
                for j in range(2):
                    s = sp * 2 + j
                    nc.tensor.matmul(psp[:, j, 0:D], ofmA[:, ts(s, 128)],
                                     sb["wp0"], start=True, stop=False)
                    nc.tensor.matmul(psp[:, j, 0:D], ofmB[:, ts(s, 128)],
                                     sb["wp1"], start=False, stop=True)
                nc.vector.tensor_add(x_t[:, ds(sp * 2, 2), 0:D],
                                     x_t[:, ds(sp * 2, 2), 0:D],
                                     psp[:, :, 0:D])

        def phase_c_mlp(sb, x_t, ynA, ynA2):
            ynAf = ynA.rearrange("p a b -> p (a b)")
            ynA2f = ynA2.rearrange("p a b -> p (a b)")
            hfm = hpool.tile([128, 6, TILE], bf16, tag="hfm", name="hfm")
            for m in range(6):
                psf = ppm.tile([128, TILE], f32, tag="med", name="psf1")
                nc.tensor.matmul(psf, sb["w10"][:, ts(m, 128)],
                                 ynAf, start=True, stop=False)
                nc.tensor.matmul(psf, sb["w11"][:, ts(m, 128)],
                                 ynA2f[0:64], start=False, stop=True)
                nc.scalar.activation(hfm[:, m], psf, GELU_FUNC)
            for sp in range(2):
                psf2 = ppm.tile([128, 2, 256], f32, tag="med", name="psf2")
                for j in range(2):
                    s = sp * 2 + j
                    for m in range(6):
                        nc.tensor.matmul(psf2[:, j, 0:D],
                                         hfm[:, m, ts(s, 128)],
                                         sb["w2m"][:, m],
                                         start=(m == 0), stop=(m == 5))
                nc.vector.tensor_add(x_t[:, ds(sp * 2, 2), 0:D],
                                     x_t[:, ds(sp * 2, 2), 0:D],
                                     psf2[:, :, 0:D])

        def store_y(it, x_t):
            nc.sync.dma_start(
                out=y_d[it].rearrange("p (s f) -> p s f", s=NSUB),
                in_=x_t[:, :, 0:D])

        # ---- slot-scheduled emission (v4 structure) ----
        # Five phases per super-unit (layer, half-of-tiles):
        # LN1 / QKV / CH / PROJ+LN2 / MLP, with unit offsets chosen so
        # DVE-heavy phases share slots with PE-heavy phases of other units
        # and exp/gelu stay batched per slot.
        GPB2 = min(8, ntiles)
        nstream = ntiles // GPB2
        US = []
        for li in range(DEPTH):
            for s in range(nstream):
                US.append((li, [s * GPB2 + t for t in range(GPB2)]))
        offs = []
        for ui, (li, _) in enumerate(US):
            s = ui % nstream if nstream > 1 else 0
            offs.append(5 * li + 4 * s)

        xts = {}
        fms = {}
        qks = {}
        ofs_ = {}
        yns = {}

        def ln1_thunks(u):
            li, tiles = u
            out = []
            for it in tiles:
                def th(li=li, it=it):
                    fms[(li, it)] = layernorm_fm(xts[it], norm_on_act=True)
                out.append(th)
            return out

        def qkv_thunks(u):
            li, tiles = u
            out = []
            for it in tiles:
                def th(li=li, it=it):
                    qks[(li, it)] = phase_a(W[li], *fms.pop((li, it)))
                out.append(th)
            return out

        def ch_thunks(u):
            li, tiles = u
            thunks = []
            for it in tiles:
                def alloc(li=li, it=it):
                    ofs_[(li, it)] = (
                        ofpool.tile([128, TILE], bf16, tag="ofA",
                                    name="ofmA"),
                        ofpool.tile([64, TILE], bf16, tag="ofB",
                                    name="ofmB"))
                thunks.append(alloc)
            chains = [(it, sp) for it in tiles for sp in range(2)]
            SKEW = 3
            pks = {}
            for ci in range(len(chains) + SKEW):
                if ci < len(chains):
                    def soft(ci=ci, li=li):
                        it, sp = chains[ci]
                        qkA, qkB, _ = qks[(li, it)]
                        pks[ci] = attn_soft(qkA, qkB, sp)
                    thunks.append(soft)
                if ci >= SKEW:
                    def outt(ci=ci, li=li):
                        it, sp = chains[ci - SKEW]
                        _, _, v_tm = qks[(li, it)]
                        attn_out(v_tm, pks.pop(ci - SKEW),
                                 ofs_[(li, it)][0], ofs_[(li, it)][1], sp)
                    thunks.append(outt)
            def drop(li=li, tiles=tiles):
                for it in tiles:
                    qks.pop((li, it))
            thunks.append(drop)
            return thunks

        def projln2_thunks(u):
            li, tiles = u
            n = len(tiles)
            thunks = []
            for i in range(n + 1):
                if i < n:
                    it = tiles[i]
                    def proj(li=li, it=it):
                        ofmA, ofmB = ofs_.pop((li, it))
                        phase_c_proj(W[li], xts[it], ofmA, ofmB)
                    thunks.append(proj)
                if 0 <= i - 1 < n:
                    it2 = tiles[i - 1]
                    def ln2(li=li, it=it2):
                        yns[(li, it)] = layernorm_fm(xts[it],
                                                     norm_on_act=False)
                    thunks.append(ln2)
            return thunks

        def mlp_thunks(u):
            li, tiles = u
            out = []
            for it in tiles:
                def th(li=li, it=it):
                    phase_c_mlp(W[li], xts[it], *yns.pop((li, it)))
                    if li == DEPTH - 1:
                        store_y(it, xts[it])
                out.append(th)
            return out

        PHASES = [ln1_thunks, qkv_thunks, ch_thunks, projln2_thunks,
                  mlp_thunks]

        from collections import defaultdict as _dd
        slots = _dd(list)
        for ui, u in enumerate(US):
            for p in range(5):
                slots[offs[ui] + p].append((p, u))

        for it in range(ntiles):
            xts[it] = load_x(it)

        def interleave(lists):
            idx = [0] * len(lists)
            total = sum(len(l) for l in lists)
            for _ in range(total):
                best, bf = None, 2.0
                for li_, l in enumerate(lists):
                    if idx[li_] < len(l):
                        f = idx[li_] / len(l)
                        if f < bf:
                            best, bf = li_, f
                lists[best][idx[best]]()
                idx[best] += 1

        for sl in sorted(slots):
            entries = slots[sl]
            entries.sort(key=lambda e: (e[0] != 2, e[0]))
            interleave([PHASES[p](u) for p, u in entries])

    nc.compile()
    _COMPILED[key] = nc
    return nc


def _ensure_ntff_hook():
    import sys, types
    if "antenv.axon_hooks" in sys.modules:
        return True
    try:
        mod = types.ModuleType("antenv.axon_hooks")
        state = {}
        mod.set_axon_ntff_profile_hook = lambda h: state.__setitem__("h", h)
        mod.get_axon_ntff_profile_hook = lambda: state.get("h")
        sys.modules["antenv.axon_hooks"] = mod
        import antenv
        antenv.axon_hooks = mod
        from trn_agent_boot.trn_boot import _ntff_profile_via_ctypes
        mod.set_axon_ntff_profile_hook(
            _ntff_profile_via_ctypes("/opt/axon/libaxon_pjrt.so"))
        return True
    except Exception as e:  # pragma: no cover
        print(f"NTFF hook shim failed: {e}")
        return False


def _run(inputs, trace=False):
    """Shard, execute on 8 cores, gather. Returns (y_full, exec_time_ns)."""
    from concourse.bass_utils import run_bass_kernel_spmd

    if trace:
        trace = _ensure_ntff_hook()

    wmap = _fold_weights(inputs)
    nc = _build_nc()

    x = np.asarray(inputs["x"], np.float32)
    pos = np.asarray(inputs["pos"], np.float32)
    w = int(np.asarray(inputs["w"]))
    order = _scanline_order(pos, w)
    x_ord = np.take_along_axis(x, order[..., None], axis=1)
    # device layout: [NTILES, 128 (token-in-sub), NSUB, DP]
    sw = np.zeros((NCORES, NTILES, 128, NSUB, DP), ml_dtypes.bfloat16)
    sw[..., 0:D] = x_ord.reshape(NCORES, NTILES, NSUB, 128, D).transpose(
        0, 1, 3, 2, 4)
    shards = sw.reshape(NCORES, NTILES, 128, NSUB * DP)

    wmap = {k: np.ascontiguousarray(v) for k, v in wmap.items()}
    in_maps = [{"x": shards[c], **wmap} for c in range(NCORES)]
    res = run_bass_kernel_spmd(nc, in_maps, core_ids=list(range(NCORES)),
                               trace=trace)
    y_ord = np.stack([np.asarray(res.results[c]["y"], np.float32)
                      for c in range(NCORES)])
    y_ord = y_ord.reshape(NCORES, NTILES, 128, NSUB, D).transpose(
        0, 1, 3, 2, 4)
    y_ord = y_ord.reshape(B, N, D)
    y = np.empty_like(y_ord)
    np.put_along_axis(y, order[..., None], y_ord, axis=1)
    return y.astype(np.float32), res.exec_time_ns


def kernel(**inputs):
    y, _ = _run(inputs, trace=False)
    return y
